# revision 35
# baseline (speedup 1.0000x reference)
"""DyConv2d (dynamic convolution with SE attention) on 8 TRN2 NeuronCores.

Reference computation (per image):
    attn = softmax(MLP(global_avg_pool(x)) / T)            # [K=4]
    y    = conv3x3(x, W) + bias                            # W: [K*128, 128, 3, 3]
    out  = sum_k attn[k] * y[k]                            # [128, 64, 64]

Key algebraic rewrite: conv is linear in the weights, so
    out = conv3x3(x, sum_k attn[k] * W_k) + sum_k attn[k] * bias_k
which cuts the conv FLOPs by 4x (one 128->128 conv per image instead of
128->512).

Sharding: data-parallel over batch, 2 images per core. The replicated
weights are laid out host-side in the transposed [k, ci, tap, co] order the
TensorE needs (lhsT), so no on-device transposes are required and the
per-tap-group weight DMAs pipeline with the attention computation.

Per-core pipeline (engine assignment keeps the PE the bottleneck):
  1. x DMA (sync HWDGE, 2 halves) -> DVE re-rounds to float32r (the PE's
     full-rate fp32 mode, ~1.5e-4 rel err) into a flat-padded layout and
     emits the SE global sum via accum_out.
  2. Wt DMA (scalar HWDGE queue) in tap-group-major order so the first
     combine group unblocks after ~1/3 of the weight bytes.
  3. Per-image SE MLP on PE (tiny, exact f32); softmax on ACT/DVE with two
     tiny DRAM bounces for the [4,1]->[1,4] transpose and the 128-partition
     attn broadcast (DRAM APs allow partition-stride-0).
  4. Per-image weight combine over k on DVE in 3 groups of 3 taps (fused
     scalar_tensor_tensor chain, final write rounds to f32r), so the conv
     starts right after group 0.
  5. Conv: flat-padded layout with row pitch 65 -> each row's right pad
     aliases the next row's left pad (zero), so every 3x3 tap is one fully
     contiguous fp32r matmul at flat offset dy*65+dx. Tap-major over groups
     of 2-3 row-blocks (PSUM banks), 9 accumulating matmuls per bank,
     N = 7*65+1 = 456 (fp32r requires even N <= 512).
  6. Eviction adds the attn-combined bias on ACT (Identity + bias AP) and
     DMAs out on alternating HWDGE queues.
"""

import sys

sys.path.insert(0, "/opt/trn_rl_repo")

import numpy as np

from concourse import bacc, mybir
import concourse.tile as tile
from concourse.bass_utils import run_bass_kernel_spmd
from concourse.tile_rust import add_dep_helper

B_TOTAL = 16
N_CORES = 8
B = B_TOTAL // N_CORES  # images per core
CI = 128
CO = 128
K = 4
H = W = 64
TEMP = 30.0
F32 = mybir.dt.float32
F32R = mybir.dt.float32r

# raster order: tap i = (i//3, i%3), matching the weff group layout
TAPS = [(i // 3, i % 3) for i in range(9)]
XPL = 65 * 66 + 4  # padded-x flat length (extra zeros absorb window overrun)
BLOCKS = [(h0, 7) for h0 in range(0, 63, 7)] + [(63, 1)]
BGROUPS = [BLOCKS[0:3], BLOCKS[3:6], BLOCKS[6:8], BLOCKS[8:10]]

_NC_CACHE = {}


def build_nc(reps=1):
    nc = bacc.Bacc("TRN2", target_bir_lowering=False)

    x_d = nc.dram_tensor("x2", [B, CI, H, W], F32, kind="ExternalInput")
    wt_d = nc.dram_tensor("weight_t", [K, CI, 9, CO], F32, kind="ExternalInput")
    bc_d = nc.dram_tensor("bias_cos", [CO, K], F32, kind="ExternalInput")
    w1t_d = nc.dram_tensor("se_w1t", [CI, 33], F32, kind="ExternalInput")
    w2t_d = nc.dram_tensor("se_w2t", [33, K], F32, kind="ExternalInput")
    b2_d = nc.dram_tensor("se_b2", [K], F32, kind="ExternalInput")
    y_d = nc.dram_tensor("y2", [B, CO, H, W], F32, kind="ExternalOutput")

    with tile.TileContext(nc) as tc:
        with (
            tc.tile_pool(name="consts", bufs=1) as consts,
            tc.tile_pool(name="ximg", bufs=2) as ximg,
            tc.tile_pool(name="weff", bufs=6) as weffp,
            tc.tile_pool(name="cmb", bufs=2) as cmbp,
            tc.tile_pool(name="sesb", bufs=2) as sesb,
            tc.tile_pool(name="ev", bufs=4) as evp,
            tc.tile_pool(name="cv", bufs=6, space="PSUM") as cvp,
            tc.tile_pool(name="tp", bufs=2, space="PSUM") as tpp,
        ):
            for _ in range(reps):
                build_body(nc, tc, consts, ximg, weffp, cmbp, sesb, evp, cvp,
                           tpp, x_d, wt_d, bc_d, w1t_d, w2t_d, b2_d, y_d)

    nc.compile()
    return nc


def build_body(nc, tc, consts, ximg, weffp, cmbp, sesb, evp, cvp, tpp,
               x_d, wt_d, bc_d, w1t_d, w2t_d, b2_d, y_d):
    pooled = consts.tile([128, B], F32, tag="pooled")
    pool_parts = consts.tile([128, B, 2], F32, tag="pool_parts")
    lg_dram = nc.dram_tensor("lg_bounce", [B, K], F32)
    attn_dram = nc.dram_tensor("attn_bounce", [B, K], F32)
    x_sb = [None, None]
    x_r = [None, None]

    def load_x(b):
        t = ximg.tile([128, H, W], F32, tag=f"x_sb{b}", name=f"x_sb{b}")
        nc.sync.dma_start(out=t[:, 0:32, :], in_=x_d[b, :, 0:32, :])
        nc.sync.dma_start(out=t[:, 32:64, :], in_=x_d[b, :, 32:64, :])
        x_sb[b] = t

    def round_image(b):
        """f32r-round x into the flat-padded layout; accumulate the SE sums."""
        xr = ximg.tile([128, XPL], F32R, tag=f"x_r{b}", name=f"x_r{b}")
        xr_rows = xr[:, 0:65 * 66].rearrange("p (r c) -> p r c", c=65)
        x_flat = x_sb[b].rearrange("p a b -> p (a b)")
        # zero the pad cells; memset can't produce float32r, so use in*0 ops
        for pad_out, pad_in in [
            (xr[:, 0:66], x_flat[:, 0:66]),            # top pad row
            (xr_rows[:, 2:65, 0], x_flat[:, 0:63]),    # left pads
            (xr[:, 65 * 65:XPL], x_flat[:, 0:69]),     # bottom pad row
        ]:
            nc.vector.tensor_scalar(
                out=pad_out, in0=pad_in, scalar1=0.0, scalar2=None,
                op0=mybir.AluOpType.mult,
            )
        for hh in (0, 1):  # round each 32-row half as its DMA lands
            nc.vector.tensor_scalar(
                out=xr_rows[:, 1 + 32 * hh:1 + 32 * (hh + 1), 1:65],
                in0=x_sb[b][:, 32 * hh:32 * (hh + 1), :],
                scalar1=1.0, scalar2=0.0,
                op0=mybir.AluOpType.mult, op1=mybir.AluOpType.add,
                accum_out=pool_parts[:, b, hh:hh + 1],
            )
        nc.vector.tensor_add(pooled[:, b:b + 1], pool_parts[:, b, 0:1],
                             pool_parts[:, b, 1:2])
        x_r[b] = xr

    # ---- weights (already [k, ci, tap, co] from the host), group-major ----
    wt = [consts.tile([128, 9, CO], F32, tag=f"wt{k}", name=f"wt{k}")
          for k in range(K)]

    def load_w_group(g):
        # one contiguous DMA per k (4.6KB/partition runs, max DMA efficiency)
        if g == 0:
            for k in range(K):
                nc.scalar.dma_start(out=wt[k], in_=wt_d[k])

    # tiny SE params first: a few KB that gate the whole attention chain
    w1t_sb = consts.tile([CI, 33], F32, tag="w1t_sb")
    nc.scalar.dma_start(out=w1t_sb, in_=w1t_d[:, :])
    w2t_sb = consts.tile([33, K], F32, tag="w2t_sb")
    nc.scalar.dma_start(out=w2t_sb, in_=w2t_d[:, :])
    b2_sb = consts.tile([K, 1], F32, tag="b2_sb")
    nc.scalar.dma_start(out=b2_sb, in_=b2_d[:].rearrange("(a b) -> a b", b=1))
    bias_cos = consts.tile([CO, K], F32, tag="bias_cos")
    nc.scalar.dma_start(out=bias_cos, in_=bc_d[:, :])
    load_x(0)
    load_w_group(0)
    round_image(0)

    cb_all = consts.tile([128, B], F32, tag="cb_all")

    def se_attn(b):
        """SE MLP + softmax for one image -> attn_bc [128, K]; cb into cb_all."""
        ps_h = tpp.tile([128, 512], F32, tag="tp", name="ps_h")[0:33, 0:1]
        nc.tensor.matmul(ps_h, w1t_sb, pooled[:, b:b + 1], start=True, stop=True)
        h_sb = sesb.tile([33, 1], F32, tag="h_sb")
        nc.scalar.activation(out=h_sb, in_=ps_h,
                             func=mybir.ActivationFunctionType.Relu,
                             scale=1.0 / (H * W))
        ps_lg = tpp.tile([128, 512], F32, tag="tp", name="ps_lg")[0:K, 0:1]
        nc.tensor.matmul(ps_lg, w2t_sb, h_sb, start=True, stop=True)
        lg_sb = sesb.tile([K, 1], F32, tag="lg_sb")
        nc.scalar.activation(out=lg_sb, in_=ps_lg,
                             func=mybir.ActivationFunctionType.Identity,
                             bias=b2_sb[:, 0:1], scale=1.0)
        # [4,1] -> [1,4] via a tiny DRAM bounce (DRAM APs are layout-free)
        nc.sync.dma_start(out=lg_dram[b], in_=lg_sb)
        lgt = sesb.tile([1, K], F32, tag="lgt")
        nc.sync.dma_start(out=lgt, in_=lg_dram[b].rearrange("(a k) -> a k", a=1))
        e_sb = sesb.tile([1, K], F32, tag="e_sb")
        nc.scalar.activation(out=e_sb, in_=lgt,
                             func=mybir.ActivationFunctionType.Exp,
                             scale=1.0 / TEMP)
        s_sb = sesb.tile([1, 1], F32, tag="s_sb")
        nc.vector.reduce_sum(out=s_sb, in_=e_sb, axis=mybir.AxisListType.X)
        r_sb = sesb.tile([1, 1], F32, tag="r_sb")
        nc.vector.reciprocal(out=r_sb, in_=s_sb)
        attn = sesb.tile([1, K], F32, tag="attn")
        nc.vector.tensor_scalar_mul(attn, e_sb, r_sb[:, 0:1])
        # broadcast to 128 partitions via DRAM bounce (partition stride 0)
        nc.sync.dma_start(out=attn_dram[b], in_=attn)
        attn_bc = sesb.tile([128, K], F32, tag="attn_bc")
        nc.sync.dma_start(out=attn_bc, in_=attn_dram[b].partition_broadcast(128))
        # combined bias cb = sum_k attn[k] * bias[k]
        tmp = sesb.tile([128, K], F32, tag="cbtmp")
        nc.vector.tensor_mul(tmp, bias_cos, attn_bc)
        nc.vector.reduce_sum(out=cb_all[:, b:b + 1], in_=tmp,
                             axis=mybir.AxisListType.X)
        return attn_bc

    def combine_group(attn_bc, g):
        """weff_g [128, 3, CO] (f32r) = sum_k attn[k] * wt[k][:, 3g:3g+3, :]"""
        sl = slice(3 * g, 3 * g + 3)
        t0 = cmbp.tile([128, 3, CO], F32, tag="cmb_t")
        nc.vector.tensor_scalar(
            out=t0, in0=wt[0][:, sl, :], scalar1=attn_bc[:, 0:1],
            scalar2=None, op0=mybir.AluOpType.mult)
        t1 = cmbp.tile([128, 3, CO], F32, tag="cmb_t")
        nc.vector.scalar_tensor_tensor(
            out=t1, in0=wt[1][:, sl, :], scalar=attn_bc[:, 1:2], in1=t0,
            op0=mybir.AluOpType.mult, op1=mybir.AluOpType.add)
        t2 = cmbp.tile([128, 3, CO], F32, tag="cmb_t")
        nc.vector.scalar_tensor_tensor(
            out=t2, in0=wt[2][:, sl, :], scalar=attn_bc[:, 2:3], in1=t1,
            op0=mybir.AluOpType.mult, op1=mybir.AluOpType.add)
        wg = weffp.tile([128, 3, CO], F32R, tag="weff")
        last = nc.vector.scalar_tensor_tensor(
            out=wg, in0=wt[3][:, sl, :], scalar=attn_bc[:, 3:4], in1=t2,
            op0=mybir.AluOpType.mult, op1=mybir.AluOpType.add)
        return wg, last

    def conv_image(b, weff_groups):
        xr = x_r[b]
        for grp in BGROUPS:
            pss = [cvp.tile([128, 512], F32, tag="cv", name=f"cv{j}")
                   for j in range(len(grp))]
            for i, (ky, kx) in enumerate(TAPS):
                lhsT = weff_groups[i // 3][:, i % 3, :]
                off = (ky - 1) * 65 + (kx - 1)
                for j, (ps, (h0, nr)) in enumerate(zip(pss, grp)):
                    n = nr * 65 + 1  # +1 keeps N even (fp32r requires it)
                    obase = (h0 + 1) * 65 + 1
                    nc.tensor.matmul(
                        ps[:, 0:n], lhsT, xr[:, obase + off:obase + off + n],
                        start=(i == 0), stop=(i == 8),
                    )
                    if i == 8:
                        # evict right away so the PSUM bank frees while the
                        # PE finishes the remaining stop-tap matmuls
                        out_sb = evp.tile([128, 7, W], F32, tag="ev",
                                          name=f"ev{j}")
                        ps_rows = ps[:, 0:455].rearrange("p (r c) -> p r c",
                                                         c=65)
                        nc.scalar.activation(
                            out=out_sb[:, 0:nr, :], in_=ps_rows[:, 0:nr, 0:64],
                            func=mybir.ActivationFunctionType.Identity,
                            bias=cb_all[:, b:b + 1], scale=1.0)
                        dma_eng = nc.sync if j % 2 == 0 else nc.scalar
                        dma_eng.dma_start(out=y_d[b, :, h0:h0 + nr, :],
                                          in_=out_sb[:, 0:nr, :])

    # image 0: SE -> combine groups -> conv (taps of group g follow combine g)
    attn0 = se_attn(0)
    w0 = [combine_group(attn0, g)[0] for g in range(3)]
    load_x(1)
    conv_image(0, w0)
    # image 1 prep (placed after image-0 combine so it can't delay it on DVE)
    round_image(1)
    attn1 = se_attn(1)
    w1 = [combine_group(attn1, g)[0] for g in range(3)]
    conv_image(1, w1)


def get_nc():
    if "nc" not in _NC_CACHE:
        _NC_CACHE["nc"] = build_nc()
    return _NC_CACHE["nc"]


def shard_inputs(x, weight, bias, se_w1, se_w2, se_b2):
    # host-side layout prep of the replicated (batch-independent) params:
    # weight -> [k, ci, tap, co] (the lhsT layout the TensorE consumes)
    w4 = np.ascontiguousarray(weight, np.float32).reshape(K, CO, CI, 3, 3)
    weight_t = np.ascontiguousarray(w4.transpose(0, 2, 3, 4, 1)
                                    .reshape(K, CI, 9, CO))
    common = dict(
        weight_t=weight_t,
        bias_cos=np.ascontiguousarray(
            np.asarray(bias, np.float32).reshape(K, CO).T),
        se_w1t=np.ascontiguousarray(np.asarray(se_w1, np.float32).T),
        se_w2t=np.ascontiguousarray(np.asarray(se_w2, np.float32).T),
        se_b2=np.ascontiguousarray(se_b2, np.float32),
    )
    return [
        dict(x2=np.ascontiguousarray(x[c * B:(c + 1) * B], np.float32), **common)
        for c in range(N_CORES)
    ]


def kernel(x, weight, bias, se_w1, se_w2, se_b2):
    nc = get_nc()
    in_maps = shard_inputs(x, weight, bias, se_w1, se_w2, se_b2)
    res = run_bass_kernel_spmd(nc, in_maps, core_ids=list(range(N_CORES)))
    return np.concatenate([r["y2"] for r in res.results], axis=0)
